# revision 9
# baseline (speedup 1.0000x reference)
"""ContextSNN (2-layer LIF spiking net, T=50) on 8 Trainium2 NeuronCores.

Strategy:
  - Data-parallel: batch B=4096 sharded 512 per core; weights replicated.
  - Host-side: spike_seq CENTERED (x - 0.5) and cast to fp16, transposed to
    [T, IN, B_shard]. Centering halves the fp16 quantization error of the
    uniform[0,1) inputs (values now span [-.5,.5) where fp16 ulp <= 2^-12),
    and the exact correction 0.5*sum_i W1[h,i] folds into the bias.
  - fc1 runs as a SINGLE fp16 matmul pass accumulated in fp32 PSUM
    (fp16xfp16 products are exact in fp32; only operand rounding remains,
    which the centering keeps at rel-err ~1.5e-2 on the spike counts).
  - LIF state kept on-chip in [H, B] layout; membrane bias folded away
    via u = mem - b/(1-beta) so each step is two scalar_tensor_tensor
    ops plus one per-partition-threshold compare.
  - fc2 (45 outputs) = 4 fp16 matmuls per step on the same PE stream.
"""
import sys
sys.path.insert(0, "/opt/trn_rl_repo")
import numpy as np
from contextlib import ExitStack

import concourse.bass as bass
import concourse.tile as tile
from concourse import bacc, mybir
from concourse.bass_utils import run_bass_kernel_spmd

T, B, IN, H, OUT = 50, 4096, 1500, 512, 45
INP = 1536          # IN padded to 12*128
NCORES = 8
BS = B // NCORES    # 512
BETA, THR = 0.9, 1.0
KT1 = INP // 128    # 12
MT1 = H // 128      # 4
KT2 = H // 128      # 4
f16 = mybir.dt.float16
f32 = mybir.dt.float32
ALU = mybir.AluOpType

_NC_CACHE = {}
_LAST_RES = None


def _build():
    if "nc" in _NC_CACHE:
        return _NC_CACHE["nc"]
    nc = bacc.Bacc("TRN2", target_bir_lowering=False, debug=False, num_devices=NCORES)

    x_d = nc.dram_tensor("x", [T, INP, BS], f16, kind="ExternalInput").ap()
    w1_d = nc.dram_tensor("w1", [INP, H], f16, kind="ExternalInput").ap()
    w2_d = nc.dram_tensor("w2", [H, OUT], f16, kind="ExternalInput").ap()
    thr1_d = nc.dram_tensor("thr1", [128, MT1], f32, kind="ExternalInput").ap()
    u1i_d = nc.dram_tensor("u1i", [128, MT1], f32, kind="ExternalInput").ap()
    thr2_d = nc.dram_tensor("thr2", [OUT, 1], f32, kind="ExternalInput").ap()
    u2i_d = nc.dram_tensor("u2i", [OUT, 1], f32, kind="ExternalInput").ap()
    out_d = nc.dram_tensor("out", [OUT, BS], f32, kind="ExternalOutput").ap()

    with tile.TileContext(nc) as tc:
        with ExitStack() as ctx:
            wpool = ctx.enter_context(tc.tile_pool(name="w", bufs=1))
            xpool = ctx.enter_context(tc.tile_pool(name="x", bufs=48))
            state = ctx.enter_context(tc.tile_pool(name="state", bufs=1))
            spk1pool = ctx.enter_context(tc.tile_pool(name="spk1", bufs=2))
            spk2pool = ctx.enter_context(tc.tile_pool(name="spk2", bufs=2))
            t1pool = ctx.enter_context(tc.tile_pool(name="t1", bufs=8))
            t2pool = ctx.enter_context(tc.tile_pool(name="t2", bufs=2))
            ps1 = ctx.enter_context(tc.tile_pool(name="ps1", bufs=6, space="PSUM"))
            ps2 = ctx.enter_context(tc.tile_pool(name="ps2", bufs=2, space="PSUM"))

            # warmup first: the warm memset + dummy matmuls depend on no DMA,
            # so the PE starts (and the HAM clock-gate warms) immediately
            # instead of queue-blocking behind DMA-dependent state inits.
            warm = state.tile([128, BS], f16, tag="warm")
            nc.vector.memset(warm[:], 0.0)
            # N=128 warmups: just enough PE activity to bridge the w1[0]/x[0]
            # DMA wait (~1.3us); the real matmul stream then keeps the HAM
            # busy-window filled until it flips to 2.4 GHz.
            for _ in range(12):
                pw = ps1.tile([128, BS], f32, tag="p1")
                nc.tensor.matmul(pw[:, 0:128], warm[:, 0:128], warm[:, 0:128],
                                 start=True, stop=True)

            # small param DMAs first so the state-init vector ops unblock early
            thr1 = wpool.tile([128, MT1], f32, tag="thr1")
            u1i = wpool.tile([128, MT1], f32, tag="u1i")
            nc.gpsimd.dma_start(thr1[:], thr1_d[:])
            nc.gpsimd.dma_start(u1i[:], u1i_d[:])
            thr2 = wpool.tile([OUT, 1], f32, tag="thr2")
            u2i = wpool.tile([OUT, 1], f32, tag="u2i")
            nc.gpsimd.dma_start(thr2[:], thr2_d[:])
            nc.gpsimd.dma_start(u2i[:], u2i_d[:])
            # one tile per k-chunk: the first matmul only waits on the k=0
            # DMA instead of all 12 (tile-granular dependencies).
            w1_t = []
            for k in range(KT1):
                wt = wpool.tile([128, H], f16, tag=f"w1{k}")
                nc.gpsimd.dma_start(wt[:], w1_d[k * 128:(k + 1) * 128, :])
                w1_t.append(wt)
            w2 = wpool.tile([128, KT2 * OUT], f16, tag="w2")
            for k in range(KT2):
                nc.gpsimd.dma_start(w2[:, k * OUT:(k + 1) * OUT], w2_d[k * 128:(k + 1) * 128, :])

            zeros1 = state.tile([128, BS], f32, tag="zeros1")
            nc.vector.memset(zeros1[:], 0.0)
            u1 = state.tile([128, MT1 * BS], f32, tag="u1")
            for m in range(MT1):
                nc.vector.tensor_scalar(
                    u1[:, m * BS:(m + 1) * BS], zeros1[:], u1i[:, m:m + 1], None, ALU.add
                )
            zeros2 = state.tile([OUT, BS], f32, tag="zeros2")
            nc.vector.memset(zeros2[:], 0.0)
            u2 = state.tile([OUT, BS], f32, tag="u2")
            nc.vector.tensor_scalar(u2[:], zeros2[:], u2i[:, 0:1], None, ALU.add)
            acc = state.tile([OUT, BS], f32, tag="acc")
            nc.vector.memset(acc[:], 0.0)

            spk1_prev = spk1pool.tile([128, MT1 * BS], f16)
            nc.vector.memset(spk1_prev[:], 0.0)
            spk2_prev = spk2pool.tile([OUT, BS], f32)
            nc.vector.memset(spk2_prev[:], 0.0)

            def emit_fc2_mms(spk1_t):
                # col-tiled pairs: chunks (0,1) run concurrently in PE column
                # groups {0,1} and {2,3} (disjoint XBUS streams), chunks (2,3)
                # accumulate onto the same two psum halves. ~233ns per pair
                # vs ~432ns sequential (HW-benched). Halves land at psum
                # partitions 0-44 and 64-108; the second half is DMA'd to a
                # base-0 sbuf tile for the LIF combine.
                p2 = ps2.tile([128, BS], f32)
                for k in range(KT2):
                    half = k % 2
                    psl = slice(64 * half, 64 * half + OUT)
                    nc.tensor.matmul(p2[psl, :], w2[:, k * OUT:(k + 1) * OUT],
                                     spk1_t[:, k * BS:(k + 1) * BS],
                                     start=(k < 2), stop=(k >= 2),
                                     tile_position=(0, 64 * half),
                                     skip_group_check=True)
                # psum->sbuf same-partition copy on the idle ACT engine, then a
                # partition-shifting SBUF->SBUF DMA brings the half to base 0
                # (engines have hardwired lane<->partition mapping; only DMA
                # can legally move data across partitions).
                sb64 = t2pool.tile([128, BS], f32, tag="sb64")
                nc.scalar.activation(sb64[64:64 + OUT, :], p2[64:64 + OUT, :],
                                     mybir.ActivationFunctionType.Copy)
                pb0 = t2pool.tile([OUT, BS], f32, tag="pb0")
                nc.sync.dma_start(pb0[:], sb64[64:64 + OUT, :])
                return p2, pb0

            def emit_lif2(p2_pair, spk2_p):
                p2, pb0 = p2_pair
                t2 = t2pool.tile([OUT, BS], f32, tag="t2")
                nc.vector.scalar_tensor_tensor(
                    t2[:], spk2_p[:], -THR, p2[0:OUT, :], ALU.mult, ALU.add
                )
                # combine stays on DVE: a GPSIMD op here costs ~1.4us and
                # stalls the in-order DVE queue (measured: 1.47us PE gap/step)
                t2b = t2pool.tile([OUT, BS], f32, tag="t2b")
                nc.vector.tensor_add(t2b[:], t2[:], pb0[:])
                nc.vector.scalar_tensor_tensor(
                    u2[:], u2[:], BETA, t2b[:], ALU.mult, ALU.add
                )
                spk2_new = spk2pool.tile([OUT, BS], f32)
                nc.vector.tensor_scalar(
                    spk2_new[:], u2[:], thr2[:, 0:1], None, ALU.is_gt
                )
                nc.gpsimd.tensor_add(acc[:], acc[:], spk2_new[:])
                return spk2_new

            for t in range(T):
                x_t = []
                for k in range(KT1):
                    xt = xpool.tile([128, BS], f16, tag="x")
                    nc.sync.dma_start(xt[:], x_d[t, k * 128:(k + 1) * 128, :])
                    x_t.append(xt)

                spk1_new = spk1pool.tile([128, MT1 * BS], f16)

                for m in range(MT1):
                    p1 = ps1.tile([128, BS], f32)
                    sl = slice(m * 128, (m + 1) * 128)
                    for k in range(KT1):
                        nc.tensor.matmul(p1[:], w1_t[k][:, sl], x_t[k][:],
                                         start=(k == 0), stop=(k == KT1 - 1))
                    if m == 0 and t > 0:
                        # previous step's fc2 matmuls slot in behind this
                        # step's first fc1 block: their spike inputs are
                        # ready, so PE never stalls on the DVE LIF chain.
                        p2_pending = emit_fc2_mms(spk1_prev)
                    msl = slice(m * BS, (m + 1) * BS)
                    t1 = t1pool.tile([128, BS], f32, tag="t1")
                    nc.vector.scalar_tensor_tensor(
                        t1[:], spk1_prev[:, msl], -THR, p1[:], ALU.mult, ALU.add
                    )
                    nc.vector.scalar_tensor_tensor(
                        u1[:, msl], u1[:, msl], BETA, t1[:], ALU.mult, ALU.add
                    )
                    nc.vector.tensor_scalar(
                        spk1_new[:, msl], u1[:, msl], thr1[:, m:m + 1], None, ALU.is_gt
                    )
                if t > 0:
                    # lif2 DVE ops stay at the m-loop tail so the DVE stream
                    # never head-of-line blocks on the fc2 psum.
                    spk2_prev = emit_lif2(p2_pending, spk2_prev)
                spk1_prev = spk1_new

            spk2_prev = emit_lif2(emit_fc2_mms(spk1_prev), spk2_prev)

            nc.sync.dma_start(out_d[:], acc[:])

    nc.compile()
    _NC_CACHE["nc"] = nc
    return nc


def kernel(spike_seq, W1, b1, W2, b2):
    global _LAST_RES
    spike_seq = np.asarray(spike_seq, dtype=np.float32)
    W1 = np.asarray(W1, dtype=np.float32)
    W2 = np.asarray(W2, dtype=np.float32)
    b1 = np.asarray(b1, dtype=np.float32)
    b2 = np.asarray(b2, dtype=np.float32)

    nc = _build()

    W1T = np.zeros((INP, H), np.float32)
    W1T[:IN] = W1.T
    w1 = W1T.astype(np.float16)
    w2 = W2.T.copy().astype(np.float16)

    # centering correction: cur1 = xc@W1.T + (b1 + 0.5*sum_i W1[h,i])
    b1c = (b1 + 0.5 * W1.sum(axis=1)).astype(np.float32)
    thr1 = (THR - b1c / (1.0 - BETA)).astype(np.float32).reshape(MT1, 128).T.copy()
    u1i = (-b1c / (1.0 - BETA)).astype(np.float32).reshape(MT1, 128).T.copy()
    thr2 = (THR - b2 / (1.0 - BETA)).astype(np.float32).reshape(OUT, 1)
    u2i = (-b2 / (1.0 - BETA)).astype(np.float32).reshape(OUT, 1)

    common = dict(w1=w1, w2=w2, thr1=thr1, u1i=u1i, thr2=thr2, u2i=u2i)

    xc_full = (spike_seq - np.float32(0.5)).astype(np.float16)

    in_maps = []
    for c in range(NCORES):
        cs, ce = c * BS, (c + 1) * BS
        x_c = np.zeros((T, INP, BS), np.float16)
        x_c[:, :IN, :] = xc_full[:, cs:ce, :].transpose(0, 2, 1)
        m = dict(common)
        m["x"] = x_c
        in_maps.append(m)

    res = run_bass_kernel_spmd(nc, in_maps, core_ids=list(range(NCORES)))
    _LAST_RES = res
    out = np.concatenate([res.results[c]["out"].T for c in range(NCORES)], axis=0)
    return out.astype(np.float32)


# revision 10
# speedup vs baseline: 1.1439x; 1.1439x over previous
"""ContextSNN (2-layer LIF spiking net, T=50) on 8 Trainium2 NeuronCores.

Strategy:
  - Data-parallel: batch B=4096 sharded 512 per core; weights replicated.
  - Host-side: spike_seq CENTERED (x - 0.5) and cast to fp16, transposed to
    [T, IN, B_shard]. Centering halves the fp16 quantization error of the
    uniform[0,1) inputs (values now span [-.5,.5) where fp16 ulp <= 2^-12),
    and the exact correction 0.5*sum_i W1[h,i] folds into the bias.
  - fc1 runs as a SINGLE fp16 matmul pass accumulated in fp32 PSUM
    (fp16xfp16 products are exact in fp32; only operand rounding remains,
    which the centering keeps at rel-err ~1.5e-2 on the spike counts).
  - LIF state kept on-chip in [H, B] layout; membrane bias folded away
    via u = mem - b/(1-beta) so each step is two scalar_tensor_tensor
    ops plus one per-partition-threshold compare.
  - fc2 (45 outputs) = 4 fp16 matmuls per step on the same PE stream.
"""
import sys
sys.path.insert(0, "/opt/trn_rl_repo")
import numpy as np
from contextlib import ExitStack

import concourse.bass as bass
import concourse.tile as tile
from concourse import bacc, mybir
from concourse.bass_utils import run_bass_kernel_spmd

T, B, IN, H, OUT = 50, 4096, 1500, 512, 45
INP = 1536          # IN padded to 12*128
NCORES = 8
BS = B // NCORES    # 512
BETA, THR = 0.9, 1.0
KT1 = INP // 128    # 12
MT1 = H // 128      # 4
KT2 = H // 128      # 4
f16 = mybir.dt.float16
f32 = mybir.dt.float32
ALU = mybir.AluOpType

_NC_CACHE = {}
_LAST_RES = None


def _build():
    if "nc" in _NC_CACHE:
        return _NC_CACHE["nc"]
    nc = bacc.Bacc("TRN2", target_bir_lowering=False, debug=False, num_devices=NCORES)

    x_d = nc.dram_tensor("x", [T, INP, BS], f16, kind="ExternalInput").ap()
    w1_d = nc.dram_tensor("w1", [INP, H], f16, kind="ExternalInput").ap()
    w2_d = nc.dram_tensor("w2", [H, OUT], f16, kind="ExternalInput").ap()
    thr1_d = nc.dram_tensor("thr1", [128, MT1], f32, kind="ExternalInput").ap()
    u1i_d = nc.dram_tensor("u1i", [128, MT1], f32, kind="ExternalInput").ap()
    thr2_d = nc.dram_tensor("thr2", [OUT, 1], f32, kind="ExternalInput").ap()
    u2i_d = nc.dram_tensor("u2i", [OUT, 1], f32, kind="ExternalInput").ap()
    out_d = nc.dram_tensor("out", [OUT, BS], f32, kind="ExternalOutput").ap()

    with tile.TileContext(nc) as tc:
        with ExitStack() as ctx:
            wpool = ctx.enter_context(tc.tile_pool(name="w", bufs=1))
            xpool = ctx.enter_context(tc.tile_pool(name="x", bufs=48))
            state = ctx.enter_context(tc.tile_pool(name="state", bufs=1))
            spk1pool = ctx.enter_context(tc.tile_pool(name="spk1", bufs=3))
            spk2pool = ctx.enter_context(tc.tile_pool(name="spk2", bufs=4))
            t1pool = ctx.enter_context(tc.tile_pool(name="t1", bufs=8))
            t2pool = ctx.enter_context(tc.tile_pool(name="t2", bufs=4))
            ps1 = ctx.enter_context(tc.tile_pool(name="ps1", bufs=5, space="PSUM"))
            ps2 = ctx.enter_context(tc.tile_pool(name="ps2", bufs=3, space="PSUM"))

            # warmup first: the warm memset + dummy matmuls depend on no DMA,
            # so the PE starts (and the HAM clock-gate warms) immediately
            # instead of queue-blocking behind DMA-dependent state inits.
            warm = state.tile([128, BS], f16, tag="warm")
            nc.vector.memset(warm[:], 0.0)
            # N=128 warmups: just enough PE activity to bridge the w1[0]/x[0]
            # DMA wait (~1.3us); the real matmul stream then keeps the HAM
            # busy-window filled until it flips to 2.4 GHz.
            for _ in range(12):
                pw = ps1.tile([128, BS], f32, tag="p1")
                nc.tensor.matmul(pw[:, 0:128], warm[:, 0:128], warm[:, 0:128],
                                 start=True, stop=True)

            # small param DMAs first so the state-init vector ops unblock early
            thr1 = wpool.tile([128, MT1], f32, tag="thr1")
            u1i = wpool.tile([128, MT1], f32, tag="u1i")
            nc.gpsimd.dma_start(thr1[:], thr1_d[:])
            nc.gpsimd.dma_start(u1i[:], u1i_d[:])
            thr2 = wpool.tile([OUT, 1], f32, tag="thr2")
            u2i = wpool.tile([OUT, 1], f32, tag="u2i")
            nc.gpsimd.dma_start(thr2[:], thr2_d[:])
            nc.gpsimd.dma_start(u2i[:], u2i_d[:])
            # one tile per k-chunk: the first matmul only waits on the k=0
            # DMA instead of all 12 (tile-granular dependencies).
            w1_t = []
            for k in range(KT1):
                wt = wpool.tile([128, H], f16, tag=f"w1{k}")
                nc.gpsimd.dma_start(wt[:], w1_d[k * 128:(k + 1) * 128, :])
                w1_t.append(wt)
            w2 = wpool.tile([128, KT2 * OUT], f16, tag="w2")
            for k in range(KT2):
                nc.gpsimd.dma_start(w2[:, k * OUT:(k + 1) * OUT], w2_d[k * 128:(k + 1) * 128, :])

            zeros1 = state.tile([128, BS], f32, tag="zeros1")
            nc.vector.memset(zeros1[:], 0.0)
            u1 = state.tile([128, MT1 * BS], f32, tag="u1")
            for m in range(MT1):
                nc.vector.tensor_scalar(
                    u1[:, m * BS:(m + 1) * BS], zeros1[:], u1i[:, m:m + 1], None, ALU.add
                )
            zeros2 = state.tile([OUT, BS], f32, tag="zeros2")
            nc.vector.memset(zeros2[:], 0.0)
            u2 = state.tile([OUT, BS], f32, tag="u2")
            nc.vector.tensor_scalar(u2[:], zeros2[:], u2i[:, 0:1], None, ALU.add)
            acc = state.tile([OUT, BS], f32, tag="acc")
            nc.vector.memset(acc[:], 0.0)

            # spk1 lives as 4 per-m-block tiles: tile-granular semaphores mean
            # fc2 chunk k waits only on its own compare, not the whole layer.
            spk1_prev = []
            for m in range(MT1):
                s = spk1pool.tile([128, BS], f16, tag=f"spk1_{m}")
                nc.vector.memset(s[:], 0.0)
                spk1_prev.append(s)
            spk2_prev = spk2pool.tile([OUT, BS], f32)
            nc.vector.memset(spk2_prev[:], 0.0)

            def emit_fc2_mms(spk1_t):
                # col-tiled pairs: chunks (0,1) run concurrently in PE column
                # groups {0,1} and {2,3} (disjoint XBUS streams), chunks (2,3)
                # accumulate onto the same two psum halves. ~233ns per pair
                # vs ~432ns sequential (HW-benched). Halves land at psum
                # partitions 0-44 and 64-108; the second half is DMA'd to a
                # base-0 sbuf tile for the LIF combine.
                p2 = ps2.tile([128, BS], f32)
                for k in range(KT2):
                    half = k % 2
                    psl = slice(64 * half, 64 * half + OUT)
                    nc.tensor.matmul(p2[psl, :], w2[:, k * OUT:(k + 1) * OUT],
                                     spk1_t[k][:],
                                     start=(k < 2), stop=(k >= 2),
                                     tile_position=(0, 64 * half),
                                     skip_group_check=True)
                # psum->sbuf same-partition copy on the idle ACT engine, then a
                # partition-shifting SBUF->SBUF DMA brings the half to base 0
                # (engines have hardwired lane<->partition mapping; only DMA
                # can legally move data across partitions).
                sb64 = t2pool.tile([128, BS], f32, tag="sb64")
                nc.scalar.activation(sb64[64:64 + OUT, :], p2[64:64 + OUT, :],
                                     mybir.ActivationFunctionType.Copy)
                pb0 = t2pool.tile([OUT, BS], f32, tag="pb0")
                nc.sync.dma_start(pb0[:], sb64[64:64 + OUT, :])
                return p2, pb0

            def emit_lif2(p2_pair, spk2_p):
                p2, pb0 = p2_pair
                t2 = t2pool.tile([OUT, BS], f32, tag="t2")
                nc.vector.scalar_tensor_tensor(
                    t2[:], spk2_p[:], -THR, p2[0:OUT, :], ALU.mult, ALU.add
                )
                # combine stays on DVE: a GPSIMD op here costs ~1.4us and
                # stalls the in-order DVE queue (measured: 1.47us PE gap/step)
                t2b = t2pool.tile([OUT, BS], f32, tag="t2b")
                nc.vector.tensor_add(t2b[:], t2[:], pb0[:])
                nc.vector.scalar_tensor_tensor(
                    u2[:], u2[:], BETA, t2b[:], ALU.mult, ALU.add
                )
                spk2_new = spk2pool.tile([OUT, BS], f32)
                nc.vector.tensor_scalar(
                    spk2_new[:], u2[:], thr2[:, 0:1], None, ALU.is_gt
                )
                nc.gpsimd.tensor_add(acc[:], acc[:], spk2_new[:])
                return spk2_new

            for t in range(T):
                x_t = []
                for k in range(KT1):
                    xt = xpool.tile([128, BS], f16, tag="x")
                    nc.sync.dma_start(xt[:], x_d[t, k * 128:(k + 1) * 128, :])
                    x_t.append(xt)

                spk1_new = [None] * MT1

                for m in range(MT1):
                    p1 = ps1.tile([128, BS], f32)
                    sl = slice(m * 128, (m + 1) * 128)
                    for k in range(KT1):
                        nc.tensor.matmul(p1[:], w1_t[k][:, sl], x_t[k][:],
                                         start=(k == 0), stop=(k == KT1 - 1))
                    if m == 1 and t > 0:
                        # previous step's fc2 matmuls slot in behind this
                        # step's second fc1 block: by then the DVE has
                        # drained the prior step's chains, so the fc2 waits
                        # are satisfied and the PE queue never stalls.
                        p2_pending = emit_fc2_mms(spk1_prev)
                    msl = slice(m * BS, (m + 1) * BS)
                    t1 = t1pool.tile([128, BS], f32, tag="t1")
                    nc.vector.scalar_tensor_tensor(
                        t1[:], spk1_prev[m][:], -THR, p1[:], ALU.mult, ALU.add
                    )
                    nc.vector.scalar_tensor_tensor(
                        u1[:, msl], u1[:, msl], BETA, t1[:], ALU.mult, ALU.add
                    )
                    sm = spk1pool.tile([128, BS], f16, tag=f"spk1_{m}")
                    nc.vector.tensor_scalar(
                        sm[:], u1[:, msl], thr1[:, m:m + 1], None, ALU.is_gt
                    )
                    spk1_new[m] = sm
                if t > 0:
                    # lif2 DVE ops stay at the m-loop tail so the DVE stream
                    # never head-of-line blocks on the fc2 psum.
                    spk2_prev = emit_lif2(p2_pending, spk2_prev)
                spk1_prev = spk1_new

            spk2_prev = emit_lif2(emit_fc2_mms(spk1_prev), spk2_prev)

            nc.sync.dma_start(out_d[:], acc[:])

    nc.compile()
    _NC_CACHE["nc"] = nc
    return nc


def kernel(spike_seq, W1, b1, W2, b2):
    global _LAST_RES
    spike_seq = np.asarray(spike_seq, dtype=np.float32)
    W1 = np.asarray(W1, dtype=np.float32)
    W2 = np.asarray(W2, dtype=np.float32)
    b1 = np.asarray(b1, dtype=np.float32)
    b2 = np.asarray(b2, dtype=np.float32)

    nc = _build()

    W1T = np.zeros((INP, H), np.float32)
    W1T[:IN] = W1.T
    w1 = W1T.astype(np.float16)
    w2 = W2.T.copy().astype(np.float16)

    # centering correction: cur1 = xc@W1.T + (b1 + 0.5*sum_i W1[h,i])
    b1c = (b1 + 0.5 * W1.sum(axis=1)).astype(np.float32)
    thr1 = (THR - b1c / (1.0 - BETA)).astype(np.float32).reshape(MT1, 128).T.copy()
    u1i = (-b1c / (1.0 - BETA)).astype(np.float32).reshape(MT1, 128).T.copy()
    thr2 = (THR - b2 / (1.0 - BETA)).astype(np.float32).reshape(OUT, 1)
    u2i = (-b2 / (1.0 - BETA)).astype(np.float32).reshape(OUT, 1)

    common = dict(w1=w1, w2=w2, thr1=thr1, u1i=u1i, thr2=thr2, u2i=u2i)

    xc_full = (spike_seq - np.float32(0.5)).astype(np.float16)

    in_maps = []
    for c in range(NCORES):
        cs, ce = c * BS, (c + 1) * BS
        x_c = np.zeros((T, INP, BS), np.float16)
        x_c[:, :IN, :] = xc_full[:, cs:ce, :].transpose(0, 2, 1)
        m = dict(common)
        m["x"] = x_c
        in_maps.append(m)

    res = run_bass_kernel_spmd(nc, in_maps, core_ids=list(range(NCORES)))
    _LAST_RES = res
    out = np.concatenate([res.results[c]["out"].T for c in range(NCORES)], axis=0)
    return out.astype(np.float32)


# revision 12
# speedup vs baseline: 1.1724x; 1.0250x over previous
"""ContextSNN (2-layer LIF spiking net, T=50) on 8 Trainium2 NeuronCores.

Strategy:
  - Data-parallel: batch B=4096 sharded 512 per core; weights replicated.
  - Host-side: spike_seq CENTERED (x - 0.5) and cast to fp16, transposed to
    [T, IN, B_shard]. Centering halves the fp16 quantization error of the
    uniform[0,1) inputs (values now span [-.5,.5) where fp16 ulp <= 2^-12),
    and the exact correction 0.5*sum_i W1[h,i] folds into the bias.
  - fc1 runs as a SINGLE fp16 matmul pass accumulated in fp32 PSUM
    (fp16xfp16 products are exact in fp32; only operand rounding remains,
    which the centering keeps at rel-err ~1.5e-2 on the spike counts).
  - LIF state kept on-chip in [H, B] layout; membrane bias folded away
    via u = mem - b/(1-beta) so each step is two scalar_tensor_tensor
    ops plus one per-partition-threshold compare.
  - fc2 (45 outputs) = 4 fp16 matmuls per step on the same PE stream.
"""
import sys
sys.path.insert(0, "/opt/trn_rl_repo")
import numpy as np
from contextlib import ExitStack

import concourse.bass as bass
import concourse.tile as tile
from concourse import bacc, mybir
from concourse.bass_utils import run_bass_kernel_spmd

T, B, IN, H, OUT = 50, 4096, 1500, 512, 45
INP = 1536          # IN padded to 12*128
NCORES = 8
BS = B // NCORES    # 512
BETA, THR = 0.9, 1.0
KT1 = INP // 128    # 12
MT1 = H // 128      # 4
KT2 = H // 128      # 4
f16 = mybir.dt.float16
f32 = mybir.dt.float32
ALU = mybir.AluOpType

_NC_CACHE = {}
_LAST_RES = None


def _build():
    if "nc" in _NC_CACHE:
        return _NC_CACHE["nc"]
    nc = bacc.Bacc("TRN2", target_bir_lowering=False, debug=False, num_devices=NCORES)

    x_d = nc.dram_tensor("x", [T, INP, BS], f16, kind="ExternalInput").ap()
    w1_d = nc.dram_tensor("w1", [INP, H], f16, kind="ExternalInput").ap()
    w2_d = nc.dram_tensor("w2", [H, OUT], f16, kind="ExternalInput").ap()
    thr1_d = nc.dram_tensor("thr1", [128, MT1], f32, kind="ExternalInput").ap()
    u1i_d = nc.dram_tensor("u1i", [128, MT1], f32, kind="ExternalInput").ap()
    thr2_d = nc.dram_tensor("thr2", [OUT, 1], f32, kind="ExternalInput").ap()
    u2i_d = nc.dram_tensor("u2i", [OUT, 1], f32, kind="ExternalInput").ap()
    out_d = nc.dram_tensor("out", [OUT, BS], f32, kind="ExternalOutput").ap()

    with tile.TileContext(nc) as tc:
        with ExitStack() as ctx:
            wpool = ctx.enter_context(tc.tile_pool(name="w", bufs=1))
            xpool = ctx.enter_context(tc.tile_pool(name="x", bufs=72))
            state = ctx.enter_context(tc.tile_pool(name="state", bufs=1))
            spk1pool = ctx.enter_context(tc.tile_pool(name="spk1", bufs=3))
            spk2pool = ctx.enter_context(tc.tile_pool(name="spk2", bufs=4))
            t1pool = ctx.enter_context(tc.tile_pool(name="t1", bufs=8))
            t2pool = ctx.enter_context(tc.tile_pool(name="t2", bufs=4))
            ps1 = ctx.enter_context(tc.tile_pool(name="ps1", bufs=4, space="PSUM"))
            ps2 = ctx.enter_context(tc.tile_pool(name="ps2", bufs=2, space="PSUM"))

            # warmup first: the warm memset + dummy matmuls depend on no DMA,
            # so the PE starts (and the HAM clock-gate warms) immediately
            # instead of queue-blocking behind DMA-dependent state inits.
            warm = state.tile([128, BS], f16, tag="warm")
            nc.vector.memset(warm[:], 0.0)
            # N=128 warmups: just enough PE activity to bridge the w1[0]/x[0]
            # DMA wait (~1.3us); the real matmul stream then keeps the HAM
            # busy-window filled until it flips to 2.4 GHz.
            for _ in range(12):
                pw = ps1.tile([128, BS], f32, tag="p1")
                nc.tensor.matmul(pw[:, 0:128], warm[:, 0:128], warm[:, 0:128],
                                 start=True, stop=True)

            # small param DMAs first so the state-init vector ops unblock early
            thr1 = wpool.tile([128, MT1], f32, tag="thr1")
            u1i = wpool.tile([128, MT1], f32, tag="u1i")
            nc.gpsimd.dma_start(thr1[:], thr1_d[:])
            nc.gpsimd.dma_start(u1i[:], u1i_d[:])
            thr2 = wpool.tile([OUT, 1], f32, tag="thr2")
            u2i = wpool.tile([OUT, 1], f32, tag="u2i")
            nc.gpsimd.dma_start(thr2[:], thr2_d[:])
            nc.gpsimd.dma_start(u2i[:], u2i_d[:])
            # one tile per k-chunk: the first matmul only waits on the k=0
            # DMA instead of all 12 (tile-granular dependencies).
            w1_t = []
            for k in range(KT1):
                wt = wpool.tile([128, H], f16, tag=f"w1{k}")
                nc.gpsimd.dma_start(wt[:], w1_d[k * 128:(k + 1) * 128, :])
                w1_t.append(wt)
            w2 = wpool.tile([128, KT2 * OUT], f16, tag="w2")
            for k in range(KT2):
                nc.gpsimd.dma_start(w2[:, k * OUT:(k + 1) * OUT], w2_d[k * 128:(k + 1) * 128, :])

            zeros1 = state.tile([128, BS], f32, tag="zeros1")
            nc.vector.memset(zeros1[:], 0.0)
            u1 = state.tile([128, MT1 * BS], f32, tag="u1")
            for m in range(MT1):
                nc.vector.tensor_scalar(
                    u1[:, m * BS:(m + 1) * BS], zeros1[:], u1i[:, m:m + 1], None, ALU.add
                )
            zeros2 = state.tile([OUT, BS], f32, tag="zeros2")
            nc.vector.memset(zeros2[:], 0.0)
            u2 = state.tile([OUT, BS], f32, tag="u2")
            nc.vector.tensor_scalar(u2[:], zeros2[:], u2i[:, 0:1], None, ALU.add)
            acc = state.tile([OUT, BS], f32, tag="acc")
            nc.vector.memset(acc[:], 0.0)

            # spk1 lives as 4 per-m-block tiles: tile-granular semaphores mean
            # fc2 chunk k waits only on its own compare, not the whole layer.
            spk1_prev = []
            for m in range(MT1):
                s = spk1pool.tile([128, BS], f16, tag=f"spk1_{m}")
                nc.vector.memset(s[:], 0.0)
                spk1_prev.append(s)
            spk2_prev = spk2pool.tile([OUT, BS], f32)
            nc.vector.memset(spk2_prev[:], 0.0)

            def emit_fc2_mms(spk1_t):
                # col-tiled pairs: chunks (0,1) run concurrently in PE column
                # groups {0,1} and {2,3} (disjoint XBUS streams), chunks (2,3)
                # accumulate onto the same two psum halves. ~233ns per pair
                # vs ~432ns sequential (HW-benched). Halves land at psum
                # partitions 0-44 and 64-108; the second half is DMA'd to a
                # base-0 sbuf tile for the LIF combine.
                p2 = ps2.tile([128, BS], f32)
                for k in range(KT2):
                    half = k % 2
                    psl = slice(64 * half, 64 * half + OUT)
                    nc.tensor.matmul(p2[psl, :], w2[:, k * OUT:(k + 1) * OUT],
                                     spk1_t[k][:],
                                     start=(k < 2), stop=(k >= 2),
                                     tile_position=(0, 64 * half),
                                     skip_group_check=True)
                # psum->sbuf same-partition copy on the idle ACT engine, then a
                # partition-shifting SBUF->SBUF DMA brings the half to base 0
                # (engines have hardwired lane<->partition mapping; only DMA
                # can legally move data across partitions).
                sb64 = t2pool.tile([128, BS], f32, tag="sb64")
                nc.scalar.activation(sb64[64:64 + OUT, :], p2[64:64 + OUT, :],
                                     mybir.ActivationFunctionType.Copy)
                pb0 = t2pool.tile([OUT, BS], f32, tag="pb0")
                nc.gpsimd.dma_start(pb0[:], sb64[64:64 + OUT, :])
                return p2, pb0

            def emit_lif2(p2_pair, spk2_p):
                p2, pb0 = p2_pair
                t2 = t2pool.tile([OUT, BS], f32, tag="t2")
                nc.vector.scalar_tensor_tensor(
                    t2[:], spk2_p[:], -THR, p2[0:OUT, :], ALU.mult, ALU.add
                )
                # combine stays on DVE: a GPSIMD op here costs ~1.4us and
                # stalls the in-order DVE queue (measured: 1.47us PE gap/step)
                t2b = t2pool.tile([OUT, BS], f32, tag="t2b")
                nc.vector.tensor_add(t2b[:], t2[:], pb0[:])
                nc.vector.scalar_tensor_tensor(
                    u2[:], u2[:], BETA, t2b[:], ALU.mult, ALU.add
                )
                spk2_new = spk2pool.tile([OUT, BS], f32)
                nc.vector.tensor_scalar(
                    spk2_new[:], u2[:], thr2[:, 0:1], None, ALU.is_gt
                )
                nc.gpsimd.tensor_add(acc[:], acc[:], spk2_new[:])
                return spk2_new

            for t in range(T):
                x_t = []
                for k in range(KT1):
                    xt = xpool.tile([128, BS], f16, tag="x")
                    nc.sync.dma_start(xt[:], x_d[t, k * 128:(k + 1) * 128, :])
                    x_t.append(xt)

                spk1_new = [None] * MT1

                for m in range(MT1):
                    p1 = ps1.tile([128, BS], f32)
                    sl = slice(m * 128, (m + 1) * 128)
                    for k in range(KT1):
                        nc.tensor.matmul(p1[:], w1_t[k][:, sl], x_t[k][:],
                                         start=(k == 0), stop=(k == KT1 - 1))
                    if m == 1 and t > 0:
                        # previous step's fc2 matmuls slot in behind this
                        # step's second fc1 block: by then the DVE has
                        # drained the prior step's chains, so the fc2 waits
                        # are satisfied and the PE queue never stalls.
                        p2_pending = emit_fc2_mms(spk1_prev)
                    msl = slice(m * BS, (m + 1) * BS)
                    t1 = t1pool.tile([128, BS], f32, tag="t1")
                    nc.vector.scalar_tensor_tensor(
                        t1[:], spk1_prev[m][:], -THR, p1[:], ALU.mult, ALU.add
                    )
                    nc.vector.scalar_tensor_tensor(
                        u1[:, msl], u1[:, msl], BETA, t1[:], ALU.mult, ALU.add
                    )
                    sm = spk1pool.tile([128, BS], f16, tag=f"spk1_{m}")
                    nc.vector.tensor_scalar(
                        sm[:], u1[:, msl], thr1[:, m:m + 1], None, ALU.is_gt
                    )
                    spk1_new[m] = sm
                if t > 0:
                    # lif2 DVE ops stay at the m-loop tail so the DVE stream
                    # never head-of-line blocks on the fc2 psum.
                    spk2_prev = emit_lif2(p2_pending, spk2_prev)
                spk1_prev = spk1_new

            # final step: plain sequential fc2 and an all-DVE LIF2 — the
            # packed-fc2 combine chain (ACT copy -> DMA -> add) and the GPSIMD
            # acc-add would sit serially on the kernel epilogue here.
            p2f = ps2.tile([OUT, BS], f32, tag="p2f")
            for k in range(KT2):
                nc.tensor.matmul(p2f[:], w2[:, k * OUT:(k + 1) * OUT],
                                 spk1_prev[k][:],
                                 start=(k == 0), stop=(k == KT2 - 1))
            t2f = t2pool.tile([OUT, BS], f32, tag="t2f")
            nc.vector.scalar_tensor_tensor(
                t2f[:], spk2_prev[:], -THR, p2f[:], ALU.mult, ALU.add
            )
            nc.vector.scalar_tensor_tensor(
                u2[:], u2[:], BETA, t2f[:], ALU.mult, ALU.add
            )
            spk2_f = spk2pool.tile([OUT, BS], f32)
            nc.vector.tensor_scalar(
                spk2_f[:], u2[:], thr2[:, 0:1], None, ALU.is_gt
            )
            nc.vector.tensor_add(acc[:], acc[:], spk2_f[:])

            nc.sync.dma_start(out_d[:], acc[:])

    nc.compile()
    _NC_CACHE["nc"] = nc
    return nc


def kernel(spike_seq, W1, b1, W2, b2):
    global _LAST_RES
    spike_seq = np.asarray(spike_seq, dtype=np.float32)
    W1 = np.asarray(W1, dtype=np.float32)
    W2 = np.asarray(W2, dtype=np.float32)
    b1 = np.asarray(b1, dtype=np.float32)
    b2 = np.asarray(b2, dtype=np.float32)

    nc = _build()

    W1T = np.zeros((INP, H), np.float32)
    W1T[:IN] = W1.T
    w1 = W1T.astype(np.float16)
    w2 = W2.T.copy().astype(np.float16)

    # centering correction: cur1 = xc@W1.T + (b1 + 0.5*sum_i W1[h,i])
    b1c = (b1 + 0.5 * W1.sum(axis=1)).astype(np.float32)
    thr1 = (THR - b1c / (1.0 - BETA)).astype(np.float32).reshape(MT1, 128).T.copy()
    u1i = (-b1c / (1.0 - BETA)).astype(np.float32).reshape(MT1, 128).T.copy()
    thr2 = (THR - b2 / (1.0 - BETA)).astype(np.float32).reshape(OUT, 1)
    u2i = (-b2 / (1.0 - BETA)).astype(np.float32).reshape(OUT, 1)

    common = dict(w1=w1, w2=w2, thr1=thr1, u1i=u1i, thr2=thr2, u2i=u2i)

    xc_full = (spike_seq - np.float32(0.5)).astype(np.float16)

    in_maps = []
    for c in range(NCORES):
        cs, ce = c * BS, (c + 1) * BS
        x_c = np.zeros((T, INP, BS), np.float16)
        x_c[:, :IN, :] = xc_full[:, cs:ce, :].transpose(0, 2, 1)
        m = dict(common)
        m["x"] = x_c
        in_maps.append(m)

    res = run_bass_kernel_spmd(nc, in_maps, core_ids=list(range(NCORES)))
    _LAST_RES = res
    out = np.concatenate([res.results[c]["out"].T for c in range(NCORES)], axis=0)
    return out.astype(np.float32)


# revision 13
# speedup vs baseline: 1.1730x; 1.0005x over previous
"""ContextSNN (2-layer LIF spiking net, T=50) on 8 Trainium2 NeuronCores.

Strategy:
  - Data-parallel: batch B=4096 sharded 512 per core; weights replicated.
  - Host-side: spike_seq CENTERED (x - 0.5) and cast to fp16, transposed to
    [T, IN, B_shard]. Centering halves the fp16 quantization error of the
    uniform[0,1) inputs (values now span [-.5,.5) where fp16 ulp <= 2^-12),
    and the exact correction 0.5*sum_i W1[h,i] folds into the bias.
  - fc1 runs as a SINGLE fp16 matmul pass accumulated in fp32 PSUM
    (fp16xfp16 products are exact in fp32; only operand rounding remains,
    which the centering keeps at rel-err ~1.5e-2 on the spike counts).
  - LIF state kept on-chip in [H, B] layout; membrane bias folded away
    via u = mem - b/(1-beta) so each step is two scalar_tensor_tensor
    ops plus one per-partition-threshold compare.
  - fc2 (45 outputs) = 4 fp16 matmuls per step on the same PE stream.
"""
import sys
sys.path.insert(0, "/opt/trn_rl_repo")
import numpy as np
from contextlib import ExitStack

import concourse.bass as bass
import concourse.tile as tile
from concourse import bacc, mybir
from concourse.bass_utils import run_bass_kernel_spmd

T, B, IN, H, OUT = 50, 4096, 1500, 512, 45
INP = 1536          # IN padded to 12*128
NCORES = 8
BS = B // NCORES    # 512
BETA, THR = 0.9, 1.0
KT1 = INP // 128    # 12
MT1 = H // 128      # 4
KT2 = H // 128      # 4
f16 = mybir.dt.float16
f32 = mybir.dt.float32
ALU = mybir.AluOpType

_NC_CACHE = {}
_LAST_RES = None


def _build():
    if "nc" in _NC_CACHE:
        return _NC_CACHE["nc"]
    nc = bacc.Bacc("TRN2", target_bir_lowering=False, debug=False, num_devices=NCORES)

    x_d = nc.dram_tensor("x", [T, INP, BS], f16, kind="ExternalInput").ap()
    w1_d = nc.dram_tensor("w1", [INP, H], f16, kind="ExternalInput").ap()
    w2_d = nc.dram_tensor("w2", [H, OUT], f16, kind="ExternalInput").ap()
    thr1_d = nc.dram_tensor("thr1", [128, MT1], f32, kind="ExternalInput").ap()
    u1i_d = nc.dram_tensor("u1i", [128, MT1], f32, kind="ExternalInput").ap()
    thr2_d = nc.dram_tensor("thr2", [OUT, 1], f32, kind="ExternalInput").ap()
    u2i_d = nc.dram_tensor("u2i", [OUT, 1], f32, kind="ExternalInput").ap()
    out_d = nc.dram_tensor("out", [OUT, BS], f32, kind="ExternalOutput").ap()

    with tile.TileContext(nc) as tc:
        with ExitStack() as ctx:
            wpool = ctx.enter_context(tc.tile_pool(name="w", bufs=1))
            xpool = ctx.enter_context(tc.tile_pool(name="x", bufs=72))
            state = ctx.enter_context(tc.tile_pool(name="state", bufs=1))
            spk1pool = ctx.enter_context(tc.tile_pool(name="spk1", bufs=3))
            spk2pool = ctx.enter_context(tc.tile_pool(name="spk2", bufs=4))
            t1pool = ctx.enter_context(tc.tile_pool(name="t1", bufs=8))
            t2pool = ctx.enter_context(tc.tile_pool(name="t2", bufs=4))
            ps1 = ctx.enter_context(tc.tile_pool(name="ps1", bufs=4, space="PSUM"))
            ps2 = ctx.enter_context(tc.tile_pool(name="ps2", bufs=2, space="PSUM"))

            # warmup first: the warm memset + dummy matmuls depend on no DMA,
            # so the PE starts (and the HAM clock-gate warms) immediately
            # instead of queue-blocking behind DMA-dependent state inits.
            warm = state.tile([128, BS], f16, tag="warm")
            nc.vector.memset(warm[:], 0.0)
            # N=128 warmups: just enough PE activity to bridge the w1[0]/x[0]
            # DMA wait (~1.3us); the real matmul stream then keeps the HAM
            # busy-window filled until it flips to 2.4 GHz.
            for _ in range(12):
                pw = ps1.tile([128, BS], f32, tag="p1")
                nc.tensor.matmul(pw[:, 0:128], warm[:, 0:128], warm[:, 0:128],
                                 start=True, stop=True)

            # small param DMAs first so the state-init vector ops unblock early
            thr1 = wpool.tile([128, MT1], f32, tag="thr1")
            u1i = wpool.tile([128, MT1], f32, tag="u1i")
            nc.gpsimd.dma_start(thr1[:], thr1_d[:])
            nc.gpsimd.dma_start(u1i[:], u1i_d[:])
            thr2 = wpool.tile([OUT, 1], f32, tag="thr2")
            u2i = wpool.tile([OUT, 1], f32, tag="u2i")
            nc.gpsimd.dma_start(thr2[:], thr2_d[:])
            nc.gpsimd.dma_start(u2i[:], u2i_d[:])
            # one tile per k-chunk: the first matmul only waits on the k=0
            # DMA instead of all 12 (tile-granular dependencies).
            w1_t = []
            for k in range(KT1):
                wt = wpool.tile([128, H], f16, tag=f"w1{k}")
                nc.gpsimd.dma_start(wt[:], w1_d[k * 128:(k + 1) * 128, :])
                w1_t.append(wt)
            w2 = wpool.tile([128, KT2 * OUT], f16, tag="w2")
            for k in range(KT2):
                nc.gpsimd.dma_start(w2[:, k * OUT:(k + 1) * OUT], w2_d[k * 128:(k + 1) * 128, :])

            zeros1 = state.tile([128, BS], f32, tag="zeros1")
            nc.vector.memset(zeros1[:], 0.0)
            u1 = state.tile([128, MT1 * BS], f32, tag="u1")
            for m in range(MT1):
                nc.vector.tensor_scalar(
                    u1[:, m * BS:(m + 1) * BS], zeros1[:], u1i[:, m:m + 1], None, ALU.add
                )
            zeros2 = state.tile([OUT, BS], f32, tag="zeros2")
            nc.vector.memset(zeros2[:], 0.0)
            u2 = state.tile([OUT, BS], f32, tag="u2")
            nc.vector.tensor_scalar(u2[:], zeros2[:], u2i[:, 0:1], None, ALU.add)
            acc = state.tile([OUT, BS], f32, tag="acc")
            nc.vector.memset(acc[:], 0.0)

            # spk1 lives as 4 per-m-block tiles: tile-granular semaphores mean
            # fc2 chunk k waits only on its own compare, not the whole layer.
            spk1_prev = []
            for m in range(MT1):
                s = spk1pool.tile([128, BS], f16, tag=f"spk1_{m}")
                nc.vector.memset(s[:], -1.0)
                spk1_prev.append(s)
            spk2_prev = spk2pool.tile([OUT, BS], f32)
            nc.vector.memset(spk2_prev[:], 0.0)

            def emit_fc2_mms(spk1_t):
                # col-tiled pairs: chunks (0,1) run concurrently in PE column
                # groups {0,1} and {2,3} (disjoint XBUS streams), chunks (2,3)
                # accumulate onto the same two psum halves. ~233ns per pair
                # vs ~432ns sequential (HW-benched). Halves land at psum
                # partitions 0-44 and 64-108; the second half is DMA'd to a
                # base-0 sbuf tile for the LIF combine.
                p2 = ps2.tile([128, BS], f32)
                for k in range(KT2):
                    half = k % 2
                    psl = slice(64 * half, 64 * half + OUT)
                    nc.tensor.matmul(p2[psl, :], w2[:, k * OUT:(k + 1) * OUT],
                                     spk1_t[k][:],
                                     start=(k < 2), stop=(k >= 2),
                                     tile_position=(0, 64 * half),
                                     skip_group_check=True)
                # psum->sbuf same-partition copy on the idle ACT engine, then a
                # partition-shifting SBUF->SBUF DMA brings the half to base 0
                # (engines have hardwired lane<->partition mapping; only DMA
                # can legally move data across partitions).
                sb64 = t2pool.tile([128, BS], f32, tag="sb64")
                nc.scalar.activation(sb64[64:64 + OUT, :], p2[64:64 + OUT, :],
                                     mybir.ActivationFunctionType.Copy)
                pb0 = t2pool.tile([OUT, BS], f32, tag="pb0")
                nc.gpsimd.dma_start(pb0[:], sb64[64:64 + OUT, :])
                return p2, pb0

            def emit_lif2(p2_pair, spk2_p):
                p2, pb0 = p2_pair
                t2 = t2pool.tile([OUT, BS], f32, tag="t2")
                nc.vector.scalar_tensor_tensor(
                    t2[:], spk2_p[:], -THR, p2[0:OUT, :], ALU.mult, ALU.add
                )
                # combine stays on DVE: a GPSIMD op here costs ~1.4us and
                # stalls the in-order DVE queue (measured: 1.47us PE gap/step)
                t2b = t2pool.tile([OUT, BS], f32, tag="t2b")
                nc.vector.tensor_add(t2b[:], t2[:], pb0[:])
                nc.vector.scalar_tensor_tensor(
                    u2[:], u2[:], BETA, t2b[:], ALU.mult, ALU.add
                )
                spk2_new = spk2pool.tile([OUT, BS], f32)
                nc.vector.tensor_scalar(
                    spk2_new[:], u2[:], thr2[:, 0:1], None, ALU.is_gt
                )
                nc.gpsimd.tensor_add(acc[:], acc[:], spk2_new[:])
                return spk2_new

            for t in range(T):
                x_t = []
                for k in range(KT1):
                    xt = xpool.tile([128, BS], f16, tag="x")
                    nc.sync.dma_start(xt[:], x_d[t, k * 128:(k + 1) * 128, :])
                    x_t.append(xt)

                spk1_new = [None] * MT1

                for m in range(MT1):
                    p1 = ps1.tile([128, BS], f32)
                    sl = slice(m * 128, (m + 1) * 128)
                    for k in range(KT1):
                        nc.tensor.matmul(p1[:], w1_t[k][:, sl], x_t[k][:],
                                         start=(k == 0), stop=(k == KT1 - 1))
                    if m == 1 and t > 0:
                        # previous step's fc2 matmuls slot in behind this
                        # step's second fc1 block: by then the DVE has
                        # drained the prior step's chains, so the fc2 waits
                        # are satisfied and the PE queue never stalls.
                        p2_pending = emit_fc2_mms(spk1_prev)
                    msl = slice(m * BS, (m + 1) * BS)
                    t1 = t1pool.tile([128, BS], f32, tag="t1")
                    nc.vector.scalar_tensor_tensor(
                        t1[:], spk1_prev[m][:], -0.5, p1[:], ALU.mult, ALU.add
                    )
                    nc.vector.scalar_tensor_tensor(
                        u1[:, msl], u1[:, msl], BETA, t1[:], ALU.mult, ALU.add
                    )
                    sm = spk1pool.tile([128, BS], f16, tag=f"spk1_{m}")
                    nc.scalar.sign(sm[:], u1[:, msl], thr1[:, m:m + 1])
                    spk1_new[m] = sm
                if t > 0:
                    # lif2 DVE ops stay at the m-loop tail so the DVE stream
                    # never head-of-line blocks on the fc2 psum.
                    spk2_prev = emit_lif2(p2_pending, spk2_prev)
                spk1_prev = spk1_new

            # final step: plain sequential fc2 and an all-DVE LIF2 — the
            # packed-fc2 combine chain (ACT copy -> DMA -> add) and the GPSIMD
            # acc-add would sit serially on the kernel epilogue here.
            p2f = ps2.tile([OUT, BS], f32, tag="p2f")
            for k in range(KT2):
                nc.tensor.matmul(p2f[:], w2[:, k * OUT:(k + 1) * OUT],
                                 spk1_prev[k][:],
                                 start=(k == 0), stop=(k == KT2 - 1))
            t2f = t2pool.tile([OUT, BS], f32, tag="t2f")
            nc.vector.scalar_tensor_tensor(
                t2f[:], spk2_prev[:], -THR, p2f[:], ALU.mult, ALU.add
            )
            nc.vector.scalar_tensor_tensor(
                u2[:], u2[:], BETA, t2f[:], ALU.mult, ALU.add
            )
            spk2_f = spk2pool.tile([OUT, BS], f32)
            nc.vector.tensor_scalar(
                spk2_f[:], u2[:], thr2[:, 0:1], None, ALU.is_gt
            )
            nc.vector.tensor_add(acc[:], acc[:], spk2_f[:])

            nc.sync.dma_start(out_d[:], acc[:])

    nc.compile()
    _NC_CACHE["nc"] = nc
    return nc


def kernel(spike_seq, W1, b1, W2, b2):
    global _LAST_RES
    spike_seq = np.asarray(spike_seq, dtype=np.float32)
    W1 = np.asarray(W1, dtype=np.float32)
    W2 = np.asarray(W2, dtype=np.float32)
    b1 = np.asarray(b1, dtype=np.float32)
    b2 = np.asarray(b2, dtype=np.float32)

    nc = _build()

    W1T = np.zeros((INP, H), np.float32)
    W1T[:IN] = W1.T
    w1 = W1T.astype(np.float16)
    # spk1 is carried as s=+/-1 (ACT Sign): spk=(s+1)/2 folds into fc2 as
    # w2' = W2/2 and b2' = b2 + 0.5*rowsum(W2) (both exact-enough in f32).
    w2 = (0.5 * W2.T).copy().astype(np.float16)
    b2c = (b2 + 0.5 * W2.sum(axis=1)).astype(np.float32)

    # centering correction: cur1 = xc@W1.T + (b1 + 0.5*sum_i W1[h,i])
    b1c = (b1 + 0.5 * W1.sum(axis=1)).astype(np.float32)
    # LIF1 state shifted by +5 to absorb the -0.5 constant from the +/-1
    # reset term; "thr1" now holds the Sign bias -(thr+5).
    thr1 = (-(THR - b1c / (1.0 - BETA) + 5.0)).astype(np.float32).reshape(MT1, 128).T.copy()
    u1i = (-b1c / (1.0 - BETA) + 5.0).astype(np.float32).reshape(MT1, 128).T.copy()
    thr2 = (THR - b2c / (1.0 - BETA)).astype(np.float32).reshape(OUT, 1)
    u2i = (-b2c / (1.0 - BETA)).astype(np.float32).reshape(OUT, 1)

    common = dict(w1=w1, w2=w2, thr1=thr1, u1i=u1i, thr2=thr2, u2i=u2i)

    xc_full = (spike_seq - np.float32(0.5)).astype(np.float16)

    in_maps = []
    for c in range(NCORES):
        cs, ce = c * BS, (c + 1) * BS
        x_c = np.zeros((T, INP, BS), np.float16)
        x_c[:, :IN, :] = xc_full[:, cs:ce, :].transpose(0, 2, 1)
        m = dict(common)
        m["x"] = x_c
        in_maps.append(m)

    res = run_bass_kernel_spmd(nc, in_maps, core_ids=list(range(NCORES)))
    _LAST_RES = res
    out = np.concatenate([res.results[c]["out"].T for c in range(NCORES)], axis=0)
    return out.astype(np.float32)
